# revision 17
# baseline (speedup 1.0000x reference)
"""Trainium2 Bass kernel for single-step decode attention.

Module: fused-QKV decode attention (B=8, T=1, S=4096, N=32 heads, H=128,
D=4096), one decode step at position time_step=2047.

Sharding: tensor-parallel over heads — each of the 8 cores handles 4 heads
(its slice of qkv_w / attn_vec_w / KV caches); x replicated.  The attn_vec
all-reduce is replaced by a host-side sum of the 8 tiny [8,4096] partials.

Only the valid prefix of the KV cache (positions 0..time_step) contributes
to the softmax (the reference masks out the rest, and masked positions
contribute exactly 0 to the result), so the kernel reads only
time_step+1 rows of each cache: the dominant memory traffic is halved.

Per-core device program (fp32 throughout):
  1. qkv projection on PE: psum[b, (qkv,n,h)] += xT_d.T @ w_d over 32
     D-chunks (x transposed host-side into xTp[p, (dchunk, b)]).
  2. RoPE on q and new k (DVE, with host-precomputed sin/cos tables);
     q also folds in the 1/sqrt(H) scale.
  3. Per (b): broadcast q across partitions via a ones-matmul; DMA the
     cache slice [s, 4heads, H] naturally ([s%128 partitions, ...]); the
     new k/v row is inserted at partition b of the final s-chunk.
  4. logits via DVE tensor_tensor_reduce (K-tile * q_rep, reduce over h);
     softmax without max-subtraction (logits are O(5), exp is safe, and
     softmax is shift-invariant): Exp+accum on ACT, partition-sum via a
     ones-matmul, reciprocal on DVE.
  5. encoded^T[h,1] = V_tile.T @ probs on PE (accumulated over s-chunks),
     scaled by 1/sum while copying into the attn_vec lhsT.
  6. attn partial [8, 4096] = encoded^T.T @ attn_vec_w on PE.
"""

import sys

if "/opt/trn_rl_repo" not in sys.path:
    sys.path.insert(0, "/opt/trn_rl_repo")

import numpy as np

B = 8
S_CACHE = 4096
N_HEADS = 32
H = 128
D = 4096
N_CORES = 8
HPC = N_HEADS // N_CORES          # heads per core = 4
CW = HPC * H                      # per-chunk free width = 512
P = 128                           # partitions
K_MASK = -2.3819763e38

_BUILD_CACHE = {}


def _build(n_old):
    """Build + compile the per-core Tile program.  n_old = number of old
    cache rows that participate (positions 0..n_old-1); position n_old is
    the freshly projected k/v."""
    import concourse.bacc as bacc
    import concourse.bass as bass
    import concourse.mybir as mybir
    import concourse.tile as tile

    f32 = mybir.dt.float32
    Alu = mybir.AluOpType
    Act = mybir.ActivationFunctionType

    full_chunks = n_old // P
    rem = n_old % P                # old rows in the last chunk
    n_chunks = full_chunks + 1     # last chunk: 1 new row + rem old rows
    if rem != P - 1:
        raise NotImplementedError(
            f"kernel requires time_step % {P} == {P - 1}, got {n_old % P}"
        )
    DCH = D // P                   # 32 contraction chunks for projections

    nc = bacc.Bacc(
        "TRN2",
        target_bir_lowering=False,
        debug=False,
        enable_asserts=False,
        num_devices=N_CORES,
    )

    xTp_t = nc.dram_tensor("xTp", [P, DCH * B], f32, kind="ExternalInput")
    kc_t = nc.dram_tensor("kc", [B, n_old, HPC, H], f32, kind="ExternalInput")
    vc_t = nc.dram_tensor("vc", [B, n_old, HPC, H], f32, kind="ExternalInput")
    qw_t = nc.dram_tensor("qw", [3, HPC, D, H], f32, kind="ExternalInput")
    aw_t = nc.dram_tensor("aw", [HPC, H, D], f32, kind="ExternalInput")
    consts_t = nc.dram_tensor("consts", [4, CW], f32, kind="ExternalInput")
    eye_t = nc.dram_tensor("eye", [B, B * P], f32, kind="ExternalInput")
    attn_t = nc.dram_tensor("attn", [B, D], f32, kind="ExternalOutput")
    knew_t = nc.dram_tensor("knew", [B, CW], f32, kind="ExternalOutput")
    vnew_t = nc.dram_tensor("vnew", [B, CW], f32, kind="ExternalOutput")

    with tile.TileContext(nc) as tc:
        with tc.tile_pool(name="singles", bufs=1) as singles:
            # --- constants / small inputs ---
            consts_sb = singles.tile([P, 4 * CW], f32, tag="consts")
            nc.sync.dma_start(
                out=consts_sb,
                in_=bass.AP(
                    tensor=consts_t, offset=0, ap=[[0, P], [1, 4 * CW]]
                ),
            )
            xTp_sb = singles.tile([P, DCH * B], f32, tag="xTp")
            nc.sync.dma_start(out=xTp_sb, in_=xTp_t.ap())
            ones_sb = singles.tile([P, P], f32, tag="ones")
            nc.vector.memset(ones_sb, 1.0)
            eye_sb = singles.tile([B, B * P], f32, tag="eye")
            nc.sync.dma_start(out=eye_sb, in_=eye_t.ap())

            qrow = singles.tile([B, CW], f32, tag="qrow")
            krow = singles.tile([B, CW], f32, tag="krow")
            vrow = singles.tile([B, CW], f32, tag="vrow")
            tmpa = singles.tile([B, CW], f32, tag="tmpa")
            tmpb = singles.tile([B, CW], f32, tag="tmpb")
            part_sums = singles.tile([P, B * HPC], f32, tag="psums")
            alhs = [
                singles.tile([P, B], f32, tag=f"alhs{n}", name=f"alhs{n}")
                for n in range(HPC)
            ]

            # --- phase 1: fused qkv projection ---
            with (
                tc.tile_pool(name="wpool", bufs=3) as wpool,
                tc.tile_pool(name="qkvps", bufs=1, space="PSUM") as qkvps_pool,
            ):
                qkv_ps = qkvps_pool.tile([B, 3 * CW], f32, tag="qkvps")
                for d in range(DCH):
                    w_sb = wpool.tile([P, 3 * CW], f32, tag="w")
                    nc.sync.dma_start(
                        out=w_sb,
                        in_=qw_t.ap()[:, :, d * P : (d + 1) * P, :].rearrange(
                            "q n p h -> p q n h"
                        ),
                    )
                    for g in range(3):
                        nc.tensor.matmul(
                            qkv_ps[:, g * CW : (g + 1) * CW],
                            xTp_sb[:, d * B : (d + 1) * B],
                            w_sb[:, g * CW : (g + 1) * CW],
                            start=(d == 0),
                            stop=(d == DCH - 1),
                            skip_group_check=True,
                        )

                # --- phase 2: rope on q and new k; copy new v (all b at once)
                nc.vector.tensor_copy(
                    out=vrow, in_=qkv_ps[:, 2 * CW : 3 * CW]
                )
                for dst, gbase, ci, si in (
                    (qrow, 0, 0, 1),
                    (krow, CW, 2, 3),
                ):
                    src = qkv_ps[:, gbase : gbase + CW]
                    src3 = src.rearrange("p (n t h) -> p n t h", t=2, h=64)
                    tmp3 = tmpa.rearrange("p (n t h) -> p n t h", t=2, h=64)
                    s3 = consts_sb[0:B, si * CW : (si + 1) * CW].rearrange(
                        "p (n t h) -> p n t h", t=2, h=64
                    )
                    # out_first = src_second * (-sin), out_second = src_first * sin
                    nc.vector.tensor_mul(
                        out=tmp3[:, :, 0, :],
                        in0=src3[:, :, 1, :],
                        in1=s3[:, :, 0, :],
                    )
                    nc.vector.tensor_mul(
                        out=tmp3[:, :, 1, :],
                        in0=src3[:, :, 0, :],
                        in1=s3[:, :, 1, :],
                    )
                    nc.vector.tensor_mul(
                        out=tmpb,
                        in0=src,
                        in1=consts_sb[0:B, ci * CW : (ci + 1) * CW],
                    )
                    nc.vector.tensor_add(out=dst, in0=tmpa, in1=tmpb)

            # --- phase 3: attention over the valid cache prefix ---
            lastoff = full_chunks * CW

            def cache_dmas(dst_tile, src_t, b):
                # chunks 0..full_chunks-1: rows c*128+p -> partition p
                if full_chunks:
                    nc.sync.dma_start(
                        out=dst_tile[:, : full_chunks * CW],
                        in_=src_t.ap()[b, : full_chunks * P].rearrange(
                            "(c p) n h -> p c n h", p=P
                        ),
                    )
                # last chunk: partition 0 = the new row (patched separately),
                # partitions 1..rem = the remaining old rows
                base = full_chunks * P
                nc.sync.dma_start(
                    out=dst_tile[1 : rem + 1, lastoff : lastoff + CW],
                    in_=src_t.ap()[b, base : base + rem].rearrange(
                        "r n h -> r (n h)"
                    ),
                )

            with (
                tc.tile_pool(name="kpool", bufs=2) as kpool,
                tc.tile_pool(name="vpool", bufs=2) as vpool,
                tc.tile_pool(name="qrep", bufs=2) as qrep_pool,
                tc.tile_pool(name="small", bufs=4) as small,
                tc.tile_pool(name="scratch", bufs=4) as scratch_pool,
                tc.tile_pool(name="qrepps", bufs=3, space="PSUM") as qrepps_pool,
                tc.tile_pool(name="encps", bufs=2, space="PSUM") as encps_pool,
                tc.tile_pool(name="sumps", bufs=2, space="PSUM") as sumps_pool,
            ):
                for b in range(B):
                    qrep_ps = qrepps_pool.tile([P, CW], f32, tag="rep_ps")
                    nc.tensor.matmul(
                        qrep_ps,
                        eye_sb[:, b * P : (b + 1) * P],
                        qrow,
                        start=True,
                        stop=True,
                    )
                    qrep = qrep_pool.tile([P, CW], f32, tag="qrep")
                    nc.vector.tensor_copy(out=qrep, in_=qrep_ps)

                    # replicate the new k/v rows the same way so row b can be
                    # read back from partition 0 (engine partition bases must
                    # be quadrant-aligned, so krow[b] is not directly usable)
                    kt = kpool.tile([P, n_chunks * CW], f32, tag="kt")
                    cache_dmas(kt, kc_t, b)
                    krep_ps = qrepps_pool.tile([P, CW], f32, tag="rep_ps")
                    nc.tensor.matmul(
                        krep_ps,
                        eye_sb[:, b * P : (b + 1) * P],
                        krow,
                        start=True,
                        stop=True,
                    )
                    nc.vector.tensor_copy(
                        out=kt[0:1, lastoff : lastoff + CW],
                        in_=krep_ps[0:1, :],
                    )
                    vt = vpool.tile([P, n_chunks * CW], f32, tag="vt")
                    cache_dmas(vt, vc_t, b)
                    vrep_ps = qrepps_pool.tile([P, CW], f32, tag="rep_ps")
                    nc.tensor.matmul(
                        vrep_ps,
                        eye_sb[:, b * P : (b + 1) * P],
                        vrow,
                        start=True,
                        stop=True,
                    )
                    nc.vector.tensor_copy(
                        out=vt[0:1, lastoff : lastoff + CW],
                        in_=vrep_ps[0:1, :],
                    )

                    for n in range(HPC):
                        idx = b * HPC + n
                        lg = small.tile([P, n_chunks], f32, tag="lg")
                        for c in range(n_chunks):
                            ttr_out = scratch_pool.tile([P, P], f32, tag="ttr")
                            nc.vector.scalar_tensor_tensor(
                                out=ttr_out,
                                in0=kt[:, c * CW + n * H : c * CW + (n + 1) * H],
                                scalar=1.0,
                                in1=qrep[:, n * H : (n + 1) * H],
                                op0=Alu.mult,
                                op1=Alu.mult,
                                accum_out=lg[:, c : c + 1],
                            )
                        pr = small.tile([P, n_chunks], f32, tag="pr")
                        nc.scalar.activation(
                            out=pr,
                            in_=lg,
                            func=Act.Exp,
                            accum_out=part_sums[:, idx : idx + 1],
                        )
                        sum_ps = sumps_pool.tile([P, 1], f32, tag="sum_ps")
                        nc.tensor.matmul(
                            sum_ps,
                            ones_sb,
                            part_sums[:, idx : idx + 1],
                            start=True,
                            stop=True,
                        )
                        rec = small.tile([P, 1], f32, tag="rec")
                        nc.vector.reciprocal(out=rec, in_=sum_ps)

                        enc_ps = encps_pool.tile([P, 1], f32, tag="enc")
                        for c in range(n_chunks):
                            nc.tensor.matmul(
                                enc_ps,
                                vt[:, c * CW + n * H : c * CW + (n + 1) * H],
                                pr[:, c : c + 1],
                                start=(c == 0),
                                stop=(c == n_chunks - 1),
                            )
                        nc.vector.tensor_scalar_mul(
                            out=alhs[n][:, b : b + 1],
                            in0=enc_ps,
                            scalar1=rec,
                        )

            # --- phase 4: attn_vec projection (partial over this core's heads)
            with (
                tc.tile_pool(name="awpool", bufs=2) as awpool,
                tc.tile_pool(name="attnps", bufs=1, space="PSUM") as attnps_pool,
                tc.tile_pool(name="outp", bufs=2) as outp,
            ):
                attn_ps = attnps_pool.tile([B, D], f32, tag="attn_ps")
                for n in range(HPC):
                    aw_sb = awpool.tile([P, D], f32, tag="aw")
                    nc.sync.dma_start(out=aw_sb, in_=aw_t.ap()[n])
                    for dc in range(D // 512):
                        nc.tensor.matmul(
                            attn_ps[:, dc * 512 : (dc + 1) * 512],
                            alhs[n],
                            aw_sb[:, dc * 512 : (dc + 1) * 512],
                            start=(n == 0),
                            stop=(n == HPC - 1),
                            skip_group_check=True,
                        )
                for dc in range(D // 512):
                    ot = outp.tile([B, 512], f32, tag="ot")
                    nc.vector.tensor_copy(
                        out=ot, in_=attn_ps[:, dc * 512 : (dc + 1) * 512]
                    )
                    nc.sync.dma_start(
                        out=attn_t.ap()[:, dc * 512 : (dc + 1) * 512], in_=ot
                    )
                nc.sync.dma_start(out=knew_t.ap(), in_=krow)
                nc.sync.dma_start(out=vnew_t.ap(), in_=vrow)

    nc.compile()
    return nc


def _get_nc(n_old):
    if n_old not in _BUILD_CACHE:
        _BUILD_CACHE[n_old] = _build(n_old)
    return _BUILD_CACHE[n_old]


def _rope_consts(pos):
    """Host-precomputed RoPE tables (f64 trig, cast to f32)."""
    h = np.arange(H // 2, dtype=np.float64)
    timescale = 10000.0 ** (2.0 * h / H)
    sinusoid = float(pos) / timescale
    sin = np.sin(sinusoid).astype(np.float32)
    cos = np.cos(sinusoid).astype(np.float32)
    qs = np.float32(H**-0.5)
    c2 = np.concatenate([cos, cos])
    s2 = np.concatenate([-sin, sin])
    cq = np.tile(c2 * qs, HPC)
    sq = np.tile(s2 * qs, HPC)
    ck = np.tile(c2, HPC)
    sk = np.tile(s2, HPC)
    return np.stack([cq, sq, ck, sk]).astype(np.float32)


def _eye_const():
    e = np.zeros((B, B * P), dtype=np.float32)
    for b in range(B):
        e[b, b * P : (b + 1) * P] = 1.0
    return e


def kernel(
    x,
    cache_k,
    cache_v,
    attn_mask,
    qkv_w,
    attn_vec_w,
    segment_pos,
    time_step,
    _trace=False,
):
    x = np.asarray(x, dtype=np.float32)
    cache_k = np.asarray(cache_k)
    cache_v = np.asarray(cache_v)
    attn_mask = np.asarray(attn_mask, dtype=np.float32)
    qkv_w = np.asarray(qkv_w, dtype=np.float32)
    attn_vec_w = np.asarray(attn_vec_w, dtype=np.float32)
    ts = int(np.asarray(time_step))
    pos = int(np.asarray(segment_pos))
    assert ts == pos, f"time_step {ts} != segment_pos {pos} unsupported"
    assert x.shape == (B, 1, D)

    # the kernel only reads the valid prefix; verify the mask matches the
    # causal-decode mask implied by time_step
    valid = attn_mask[0, 0] >= K_MASK * 0.5
    n_valid = int(valid.sum())
    assert n_valid == ts + 1 and valid[: ts + 1].all(), "non-causal mask"
    assert (attn_mask == attn_mask[0, 0]).all(), "mask differs across batch"

    n_old = ts  # old cache rows 0..ts-1; position ts is the new k/v

    nc = _get_nc(n_old)

    from concourse import bass_utils

    x2 = x.reshape(B, D)
    xTp = np.ascontiguousarray(
        x2.reshape(B, D // P, P).transpose(2, 1, 0).reshape(P, (D // P) * B)
    ).astype(np.float32)
    consts = _rope_consts(pos)
    eye = _eye_const()
    in_maps = []
    for c in range(N_CORES):
        hs = slice(c * HPC, (c + 1) * HPC)
        in_maps.append(
            dict(
                xTp=xTp,
                kc=np.ascontiguousarray(cache_k[:, :n_old, hs, :]),
                vc=np.ascontiguousarray(cache_v[:, :n_old, hs, :]),
                qw=np.ascontiguousarray(qkv_w[:, hs]),
                aw=np.ascontiguousarray(attn_vec_w[hs]),
                consts=consts,
                eye=eye,
            )
        )

    res = bass_utils.run_bass_kernel_spmd(
        nc, in_maps, core_ids=list(range(N_CORES)), trace=_trace
    )

    attn_out = np.zeros((B, D), dtype=np.float32)
    out_k = cache_k.copy()
    out_v = cache_v.copy()
    for c in range(N_CORES):
        r = res.results[c]
        attn_out += r["attn"]
        hs = slice(c * HPC, (c + 1) * HPC)
        out_k[:, ts, hs, :] = r["knew"].reshape(B, HPC, H)
        out_v[:, ts, hs, :] = r["vnew"].reshape(B, HPC, H)

    out = (out_k, out_v, attn_out.reshape(B, 1, D))
    if _trace:
        return out, res
    return out
